# Initial kernel scaffold
#
"""Trainium2 Bass kernel for an MNIST-style CNN (conv1->relu->conv2->relu->
maxpool2x2->fc1->relu->fc2), data-parallel over 8 NeuronCores.

Per-core mapping (256 images each):
 - conv1: K=18 matmuls over x3[(block, y, dx)] (dx-interleaved input built by
   DMA, blocks quadrant-aligned so four row-groups run concurrently via
   tile_position), lhsT is a block-Toeplitz [18,(j,oc)=128] covering 4 output
   rows per matmul.
 - conv1 PSUM evicted lane-aligned (ACT, fused ReLU+bias, bf16 cast) into
   h1j[(j=y%4, ic) 128 parts, (img, yblk, x)].
 - conv2: dx-offset matmuls contracting (dy, ic) straight out of h1j.
   Even output rows: one K=128 matmul per dx (zero-padded Toeplitz weights);
   odd rows: two K=64 matmuls (the 3-row window wraps the mod-4 ring).
   2-way column tiling puts an output row PAIR in each PSUM tile.
 - maxpool+bias+relu fused into PSUM eviction: ACT evicts odd-x columns
   (ReLU+bias), DVE scalar_tensor_tensor does (even+bias) max odd, DVE
   tensor_max reduces the row pair into the pooled feature buffer.
 - fc1 with 2x row tiling (K=64 halves on partitions 0:64/64:128), fc2 plain.
"""

import os
import numpy as np
import ml_dtypes

import concourse.bass as bass
import concourse.mybir as mybir
from concourse import bacc
from concourse.tile import TileContext
from concourse.bass_utils import run_bass_kernel_spmd

BF16 = mybir.dt.bfloat16
F32 = mybir.dt.float32
ALU = mybir.AluOpType
ACTF = mybir.ActivationFunctionType

N_CORES = 8
B_CORE = 256
G = 32                      # images per chunk
NCHUNK = B_CORE // G
W1OUT = 26
W2OUT = 24
P2 = 12

_cache = {}


def _conv2_chain(y_out):
    """[(wname, yblk)] for one conv2 output row; all pieces K=128 @ base 0."""
    yb, jj = divmod(y_out, 4)
    if jj == 0:
        return [("w2e0", yb)]
    if jj == 1:
        return [("w2e1", yb)]
    if jj == 2:
        return [("w2o1a", yb), ("w2o1b", yb + 1)]
    return [("w2o2a", yb), ("w2o2b", yb + 1)]



def _fc_body(nc, tc, fpp, fp, fc1l_sb, fc1b_sb, fc2l_sb, fc2b_sb, b2v_sb,
             pooled_v, yout):
    B_CORE_ = pooled_v.shape[2]
    psA = fpp.tile([128, B_CORE_], F32, tag="psA")
    psB = fpp.tile([128, B_CORE_], F32, tag="psB")
    for h in range(2):
        ps = psA if h == 0 else psB
        for qq in range(72):
            qg = h * 72 + qq
            nc.tensor.matmul(
                ps[:, :],
                fc1l_sb[64 * h:64 * h + 64, 128 * qq:128 * qq + 128],
                pooled_v[64 * h:64 * h + 64, qg, :],
                start=(qq == 0), stop=(qq == 71),
                tile_position=(64 * h, 0))
    sA = fp.tile([128, B_CORE_], F32, tag="sA")
    nc.scalar.copy(sA[:, :], psA[:, :])
    uu = fp.tile([128, B_CORE_], F32, tag="uu")
    nc.vector.scalar_tensor_tensor(
        out=uu[:, :], in0=psB[:, :], scalar=fc1b_sb[:, :],
        in1=sA[:, :], op0=ALU.add, op1=ALU.add)
    fc1o = fp.tile([128, B_CORE_], BF16, tag="fc1o")
    nc.vector.tensor_scalar(out=fc1o[:, :], in0=uu[:, :],
                            scalar1=0.0, scalar2=None, op0=ALU.max)
    psL = fpp.tile([10, B_CORE_], F32, tag="psL")
    nc.tensor.matmul(psL[:, :], fc2l_sb[:, :], fc1o[:, :],
                     start=True, stop=True)
    logit = fp.tile([10, B_CORE_], F32, tag="logit")
    nc.scalar.activation(logit[:, :], psL[:, :], ACTF.Identity,
                         bias=fc2b_sb[:, :])
    nc.sync.dma_start(out=yout[:, :], in_=logit[:, :])


def _build_nc():
    stage = int(os.environ.get("K_STAGE", "3"))
    sub = os.environ.get("K_SUB", "full")  # mm-even | mm-all | full
    nchunk = int(os.environ.get("K_NCHUNK", str(NCHUNK)))
    nc = bacc.Bacc("TRN2", target_bir_lowering=False, debug=False,
                   num_devices=N_CORES)

    xin = nc.dram_tensor("xin", [B_CORE, 28, 28], F32, kind="ExternalInput")
    w1b4 = nc.dram_tensor("w1b4", [128, 128], BF16, kind="ExternalInput")
    wconv2 = {
        n: nc.dram_tensor(n, [128, 192], BF16, kind="ExternalInput")
        for n in ("w2e0", "w2e1", "w2o1a", "w2o1b", "w2o2a", "w2o2b")
    }
    b1v = nc.dram_tensor("b1v", [128, 1], F32, kind="ExternalInput")
    b2v = nc.dram_tensor("b2v", [128, 1], F32, kind="ExternalInput")
    fc1l = nc.dram_tensor("fc1l", [128, 9216], BF16, kind="ExternalInput")
    fc1bv = nc.dram_tensor("fc1bv", [128, 1], F32, kind="ExternalInput")
    fc2l = nc.dram_tensor("fc2l", [128, 10], BF16, kind="ExternalInput")
    fc2bv = nc.dram_tensor("fc2bv", [10, 1], F32, kind="ExternalInput")
    yout = nc.dram_tensor("yout", [10, B_CORE], F32, kind="ExternalOutput")

    with TileContext(nc) as tc:
        with tc.tile_pool(name="wpool", bufs=1) as wp, \
             tc.tile_pool(name="persist", bufs=1) as pers:
            def load_w(dram, shape, dtype=BF16, tag=None):
                t = wp.tile(shape, dtype, tag=tag or dram.name)
                nc.sync.dma_start(out=t[:, :], in_=dram[:, :])
                return t

            w1b4_sb = load_w(w1b4, [128, 128])
            w2sb = {n: load_w(d, [128, 192]) for n, d in wconv2.items()}
            b1v_sb = load_w(b1v, [128, 1], F32)
            b2v_sb = load_w(b2v, [128, 1], F32)
            fc1l_sb = load_w(fc1l, [128, 9216])
            fc1b_sb = load_w(fc1bv, [128, 1], F32)
            fc2l_sb = load_w(fc2l, [128, 10])
            fc2b_sb = load_w(fc2bv, [10, 1], F32)

            pooled = pers.tile([128, 144 * B_CORE], BF16, tag="pooled")
            pooled_v = pooled.rearrange("p (q i) -> p q i", i=B_CORE)

            with tc.tile_pool(name="conv_sb", bufs=2) as cp, \
                 tc.tile_pool(name="evict_sb", bufs=6) as ep, \
                 tc.tile_pool(name="ps1", bufs=2, space="PSUM") as pp1, \
                 tc.tile_pool(name="ps2", bufs=4, space="PSUM") as pp2:
                for c in range(nchunk):
                    i0 = c * G
                    xin_r = xin.rearrange("b h w -> h b w")
                    # x3a holds conv1 blocks 0-3 (quadrants 0-3), x3b blocks 4-6
                    x3a = cp.tile([128, G * W1OUT], BF16, tag="x3a")
                    x3b = cp.tile([128, G * W1OUT], BF16, tag="x3b")
                    # block 6 has only 4 input rows; zero its quadrant first
                    nc.gpsimd.memset(x3b[64:96, :], 0.0)
                    for b in range(7):
                        t3 = (x3a if b < 4 else x3b)
                        s = b % 4
                        nrow = 6 if b < 6 else 4
                        for dx in range(3):
                            p0 = 32 * s + 6 * dx
                            nc.gpsimd.dma_start(
                                out=t3[p0:p0 + nrow, :],
                                in_=xin_r[4 * b:4 * b + nrow, i0:i0 + G,
                                          dx:dx + W1OUT])

                    # ---- conv1 + eviction to h1j
                    h1j = cp.tile([128, G * 7 * W1OUT], BF16, tag="h1j")
                    h1v = h1j.rearrange("p (i y x) -> p i y x", y=7, x=W1OUT)
                    # rows 26,27 slots (j=2,3 of yblk 6) are read zero-weighted
                    nc.gpsimd.memset(h1v[64:128, :, 6, :], 0.0)
                    for b in range(7):
                        t3 = (x3a if b < 4 else x3b)
                        s = b % 4
                        kk = 18 if b < 6 else 16
                        mm = 128 if b < 6 else 64
                        x3v = t3[32 * s:32 * s + kk, :].rearrange(
                            "k (i x) -> k i x", x=W1OUT)
                        for H in range(2):
                            ps1 = pp1.tile([128, 16 * W1OUT], F32, tag="ps1")
                            nc.tensor.matmul(
                                ps1[0:mm, :],
                                w1b4_sb[32 * s:32 * s + kk, 0:mm],
                                x3v[:, 16 * H:16 * H + 16, :],
                                start=True, stop=True,
                                tile_position=(32 * s, 0))
                            nc.scalar.activation(
                                h1v[0:mm, 16 * H:16 * H + 16, b, :],
                                ps1[0:mm, :], ACTF.Relu, bias=b1v_sb[0:mm, :])

                    if stage < 2:
                        continue
                    # ---- conv2 + fused pool eviction (even-row pairs first,
                    # then odd: keeps PE in one tiling mode per group)
                    ylist = [0, 2, 4, 6, 8, 10, 1, 3, 5, 7, 9, 11]
                    if sub == "mm-even":
                        ylist = [0, 2, 4, 6, 8, 10]
                    elif sub == "mm-odd":
                        ylist = [1, 3, 5, 7, 9, 11]
                    for Y in ylist:
                        ps2h0 = pp2.tile([128, 16 * W2OUT], F32, tag="ps2")
                        ps2h1 = pp2.tile([128, 16 * W2OUT], F32, tag="ps2")
                        ps2 = [ps2h0, ps2h1]
                        for t in range(2):
                            chain = _conv2_chain(2 * Y + t)
                            for H in range(2):
                                n_mm = 3 * len(chain)
                                k = 0
                                for dx in range(3):
                                    for (wn, ybk) in chain:
                                        nc.tensor.matmul(
                                            ps2[H][64 * t:64 * t + 64, :],
                                            w2sb[wn][:, 64 * dx:64 * dx + 64],
                                            h1v[:, 16 * H:16 * H + 16, ybk,
                                                dx:dx + W2OUT],
                                            start=(k == 0), stop=(k == n_mm - 1),
                                            tile_position=(0, 64 * t))
                                        k += 1
                        if sub != "full":
                            for H in range(2):
                                junk = ep.tile([128, 16 * W2OUT], BF16, tag="junk")
                                nc.scalar.copy(junk[:, :], ps2[H][:, :])
                            continue
                        for H in range(2):
                            pse = ps2[H].rearrange("p (i x2 two) -> p i x2 two",
                                                   two=2, x2=P2)
                            odd = ep.tile([128, 16 * P2], BF16, tag="odd")
                            oddv = odd.rearrange("p (i x) -> p i x", x=P2)
                            nc.scalar.activation(
                                oddv[:, :, :], pse[:, :, :, 1], ACTF.Relu,
                                bias=b2v_sb[:, :])
                            u0 = ep.tile([64, 16 * P2], BF16, tag="u0")
                            u1 = ep.tile([64, 16 * P2], BF16, tag="u1")
                            nc.vector.scalar_tensor_tensor(
                                out=u0.rearrange("p (i x) -> p i x", x=P2)[:, :, :],
                                in0=pse[0:64, :, :, 0], scalar=b2v_sb[0:64, :],
                                in1=oddv[0:64, :, :],
                                op0=ALU.add, op1=ALU.max)
                            nc.vector.scalar_tensor_tensor(
                                out=u1.rearrange("p (i x) -> p i x", x=P2)[:, :, :],
                                in0=pse[64:128, :, :, 0], scalar=b2v_sb[64:128, :],
                                in1=oddv[64:128, :, :],
                                op0=ALU.add, op1=ALU.max)
                            nc.vector.tensor_max(
                                pooled_v[0:64, Y * P2:(Y + 1) * P2,
                                         i0 + 16 * H:i0 + 16 * H + 16],
                                u0.rearrange("p (i x) -> p x i", x=P2)[:, :, :],
                                u1.rearrange("p (i x) -> p x i", x=P2)[:, :, :])
                    nc.vector.tensor_copy(
                        pooled_v[64:128, :, i0:i0 + G],
                        pooled_v[0:64, :, i0:i0 + G])

            # ---- fc1 (2x row tiling) + fc2
            with tc.tile_pool(name="fc_sb", bufs=1) as fp, \
                 tc.tile_pool(name="fc_ps", bufs=1, space="PSUM") as fpp:
                if stage < 3:
                    zz = fp.tile([10, B_CORE], F32, tag="zz")
                    nc.gpsimd.memset(zz[:, :], 0.0)
                    if stage == 2:
                        nc.vector.tensor_copy(zz[0:10, 0:144],
                                              pooled_v[0:10, :, 0])
                    nc.sync.dma_start(out=yout[:, :], in_=zz[:, :])
                if stage >= 3:
                    _fc_body(nc, tc, fpp, fp, fc1l_sb, fc1b_sb, fc2l_sb, fc2b_sb,
                             b2v_sb, pooled_v, yout)

    nc.compile()
    return nc


def _prep_weights(w1, b1, w2, b2, fc1_w, fc1_b, fc2_w, fc2_b):
    # conv1 lhsT, replicated in each partition quadrant:
    # w1b4[32s + 3r + dx, 32j + oc] = w1[oc, r-j, dx]
    w1b4 = np.zeros((128, 128), np.float32)
    for s in range(4):
        for r in range(6):
            for dx in range(3):
                for j in range(4):
                    dy = r - j
                    if 0 <= dy <= 2:
                        w1b4[32 * s + 6 * dx + r, 32 * j:32 * j + 32] = \
                            w1[:, 0, dy, dx]

    # conv2 lhsT variants [128=(jslot,ic), 192=(dx,oc)]; dy per jslot or zeroed
    def w2build(dy_by_slot):
        m = np.zeros((128, 192), np.float32)
        for js, dy in enumerate(dy_by_slot):
            if dy is None:
                continue
            for dx in range(3):
                m[32 * js:32 * js + 32, 64 * dx:64 * dx + 64] = w2[:, :, dy, dx].T
        return m

    wts = {
        "w2e0": w2build([0, 1, 2, None]),
        "w2e1": w2build([None, 0, 1, 2]),
        "w2o1a": w2build([None, None, 0, 1]),
        "w2o1b": w2build([2, None, None, None]),
        "w2o2a": w2build([None, None, None, 0]),
        "w2o2b": w2build([1, 2, None, None]),
    }

    b1v = np.tile(b1, 4).reshape(128, 1).astype(np.float32)
    b2v = np.tile(b2, 2).reshape(128, 1).astype(np.float32)
    fc1l = np.zeros((128, 9216), np.float32)
    for h in range(2):
        for qq in range(72):
            qg = h * 72 + qq
            blk = fc1_w[:, qg::144]            # [128 m, 64 koc]
            fc1l[64 * h:64 * h + 64, 128 * qq:128 * qq + 128] = blk.T
    bf = ml_dtypes.bfloat16
    out = {
        "w1b4": w1b4.astype(bf),
        "b1v": b1v, "b2v": b2v,
        "fc1l": fc1l.astype(bf),
        "fc1bv": fc1_b.reshape(128, 1).astype(np.float32),
        "fc2l": fc2_w.T.astype(bf),
        "fc2bv": fc2_b.reshape(10, 1).astype(np.float32),
    }
    for n, m in wts.items():
        out[n] = m.astype(bf)
    return out


def kernel(x, w1, b1, w2, b2, fc1_w, fc1_b, fc2_w, fc2_b, _trace=False):
    x = np.asarray(x, np.float32)
    wts = _prep_weights(np.asarray(w1, np.float32), np.asarray(b1, np.float32),
                        np.asarray(w2, np.float32), np.asarray(b2, np.float32),
                        np.asarray(fc1_w, np.float32), np.asarray(fc1_b, np.float32),
                        np.asarray(fc2_w, np.float32), np.asarray(fc2_b, np.float32))
    if "nc" not in _cache:
        _cache["nc"] = _build_nc()
    nc = _cache["nc"]
    in_maps = []
    for core in range(N_CORES):
        m = dict(wts)
        m["xin"] = np.ascontiguousarray(
            x[core * B_CORE:(core + 1) * B_CORE, 0], np.float32)
        in_maps.append(m)
    kw = {}
    if _trace:
        kw = dict(trace=True, trace_cores=[0])
    res = run_bass_kernel_spmd(nc, in_maps, core_ids=list(range(N_CORES)), **kw)
    out = np.concatenate([r["yout"].T for r in res.results], axis=0)
    _cache["last_result"] = res
    return out.astype(np.float32)



# revision 10
# speedup vs baseline: 1.5179x; 1.5179x over previous
"""Trainium2 Bass kernel for an MNIST-style CNN (conv1->relu->conv2->relu->
maxpool2x2->fc1->relu->fc2), data-parallel over 8 NeuronCores.

Per-core mapping (256 images, 8 chunks of 32):
 - x is pre-Toeplitzed on the host into [128=(blk%4, dx, row), chunk, img*26]
   bf16 so each chunk's conv1 input is ONE contiguous HWDGE DMA.
 - conv1: K=18 matmuls, blocks quadrant-aligned, 4-way row-tiled.
 - h1 stored mod-4-ring [(j=y%4, ic), (img, yblk, x)]; a phase-shifted copy
   h1k [(j2=(y+2)%4, ic), ...] is built by SBUF->SBUF DMA so every pool-pair
   of output rows is K=128-aligned.
 - conv2: per pool-pair k, 3 accumulating full-array matmuls (one per dx)
   with block-Toeplitz lhsT [128=(q,ic), 128=(t,oc)]; even pairs read h1j,
   odd pairs read h1k.
 - Software-pipelined emission: chunk c+1's conv1 (and the phase copy) is
   emitted between chunk c's even-pair and odd-pair conv2 groups, so the
   PE stream stays dense (HAM warm) and the copy DMA has a full chunk of
   slack before its consumers.
 - pool fused into eviction: ACT does odd-x (ReLU+bias), DVE STT does
   even-x max, DVE tensor_max folds the row pair across partition halves
   into pooled[(Y%2, oc), img, (Y//2, x)] (natural layout, no transposes).
 - fc1: K=128=(Y-parity, oc), 72 accumulating matmuls of N=256; fc2 plain.
"""

import os
import numpy as np
import ml_dtypes

import concourse.bass as bass
import concourse.mybir as mybir
from concourse import bacc
from concourse.tile import TileContext
from concourse.bass_utils import run_bass_kernel_spmd

BF16 = mybir.dt.bfloat16
F32 = mybir.dt.float32
ALU = mybir.AluOpType
ACTF = mybir.ActivationFunctionType

N_CORES = 8
B_CORE = 256
G = 32                      # images per chunk
NCHUNK = B_CORE // G
W1OUT = 26
W2OUT = 24
P2 = 12

_cache = {}


def _build_nc():
    n_dve_evict = int(os.environ.get("K_DVE_EVICT", "0"))
    nc = bacc.Bacc("TRN2", target_bir_lowering=False, debug=False,
                   num_devices=N_CORES)

    xina = nc.dram_tensor("xina", [128, NCHUNK * G * W1OUT], BF16,
                          kind="ExternalInput")
    xinb = nc.dram_tensor("xinb", [128, NCHUNK * G * W1OUT], BF16,
                          kind="ExternalInput")
    w1b4 = nc.dram_tensor("w1b4", [128, 128], BF16, kind="ExternalInput")
    w2t = [nc.dram_tensor(f"w2t{dx}", [128, 128], BF16, kind="ExternalInput")
           for dx in range(3)]
    b1v = nc.dram_tensor("b1v", [128, 1], F32, kind="ExternalInput")
    b2v = nc.dram_tensor("b2v", [128, 1], F32, kind="ExternalInput")
    fc1l = nc.dram_tensor("fc1l", [128, 9216], BF16, kind="ExternalInput")
    fc1bv = nc.dram_tensor("fc1bv", [128, 1], F32, kind="ExternalInput")
    fc2l = nc.dram_tensor("fc2l", [128, 10], BF16, kind="ExternalInput")
    fc2bv = nc.dram_tensor("fc2bv", [10, 1], F32, kind="ExternalInput")
    yout = nc.dram_tensor("yout", [10, B_CORE], F32, kind="ExternalOutput")

    xina_v = xina.rearrange("p (c q) -> p c q", c=NCHUNK)
    xinb_v = xinb.rearrange("p (c q) -> p c q", c=NCHUNK)

    with TileContext(nc) as tc:
        with tc.tile_pool(name="wpool", bufs=1) as wp, \
             tc.tile_pool(name="persist", bufs=1) as pers:
            def load_w(dram, shape, dtype=BF16, tag=None):
                t = wp.tile(shape, dtype, tag=tag or dram.name)
                nc.sync.dma_start(out=t[:, :], in_=dram[:, :])
                return t

            w1b4_sb = load_w(w1b4, [128, 128])
            w2t_sb = [load_w(d, [128, 128]) for d in w2t]
            b1v_sb = load_w(b1v, [128, 1], F32)
            b2v_sb = load_w(b2v, [128, 1], F32)
            fc1l_sb = load_w(fc1l, [128, 9216])
            fc1b_sb = load_w(fc1bv, [128, 1], F32)
            fc2l_sb = load_w(fc2l, [128, 10])
            fc2b_sb = load_w(fc2bv, [10, 1], F32)

            # pooled[(Y%2)*64+oc, (img, qp=(Y//2)*12+x)]
            pooled = pers.tile([128, B_CORE * 72], BF16, tag="pooled")
            pooled_v = pooled.rearrange("p (i q) -> p i q", q=72)

            with tc.tile_pool(name="x3_sb", bufs=2) as cp, \
                 tc.tile_pool(name="h1_sb", bufs=2) as hp, \
                 tc.tile_pool(name="evict_sb", bufs=4) as ep, \
                 tc.tile_pool(name="ps1", bufs=2, space="PSUM") as pp1, \
                 tc.tile_pool(name="ps2", bufs=3, space="PSUM") as pp2:

                def load_x3(c):
                    x3a = cp.tile([128, G * W1OUT], BF16, tag="x3a")
                    x3b = cp.tile([128, G * W1OUT], BF16, tag="x3b")
                    nc.sync.dma_start(out=x3a[:, :], in_=xina_v[:, c, :])
                    nc.sync.dma_start(out=x3b[:, :], in_=xinb_v[:, c, :])
                    return x3a, x3b

                def alloc_h1():
                    h1j = hp.tile([128, G * 7 * W1OUT], BF16, tag="h1j")
                    h1k = hp.tile([128, G * 7 * W1OUT], BF16, tag="h1k")
                    h1v = h1j.rearrange("p (i y x) -> p i y x", y=7, x=W1OUT)
                    h1kv = h1k.rearrange("p (i y x) -> p i y x", y=7, x=W1OUT)
                    return h1v, h1kv

                def conv1_blocks(x3a, x3b, h1v, blocks):
                    ne = 0
                    for H in range(2):
                        for b in blocks:
                            t3 = (x3a if b < 4 else x3b)
                            s = b % 4
                            kk = 18 if b < 6 else 16
                            mm = 128 if b < 6 else 64
                            x3v = t3[32 * s:32 * s + kk, :].rearrange(
                                "k (i x) -> k i x", x=W1OUT)
                            ps1 = pp1.tile([128, 16 * W1OUT], F32, tag="ps1")
                            nc.tensor.matmul(
                                ps1[0:mm, :],
                                w1b4_sb[32 * s:32 * s + kk, 0:mm],
                                x3v[:, 16 * H:16 * H + 16, :],
                                start=True, stop=True,
                                tile_position=(32 * s, 0))
                            dst = h1v[0:mm, 16 * H:16 * H + 16, b, :]
                            if ne < n_dve_evict:
                                nc.vector.tensor_scalar(
                                    out=dst, in0=ps1[0:mm, :],
                                    scalar1=b1v_sb[0:mm, :], scalar2=0.0,
                                    op0=ALU.add, op1=ALU.max)
                            else:
                                nc.scalar.activation(
                                    dst, ps1[0:mm, :], ACTF.Relu,
                                    bias=b1v_sb[0:mm, :])
                            ne += 1

                def copy_phase(h1v, h1kv):
                    # SBUF->SBUF DMA on the sync HWDGE ring
                    nc.sync.dma_start(
                        out=h1kv[0:64, :, 1:7, :], in_=h1v[64:128, :, 0:6, :])
                    nc.sync.dma_start(
                        out=h1kv[64:128, :, 0:7, :], in_=h1v[0:64, :, 0:7, :])

                def conv2_half(c, h1v, h1kv, ph, grp):
                    i0 = c * G
                    src = h1v if ph == 0 else h1kv
                    for H in range(2):
                        ps2 = pp2.tile([128, 1024], F32, tag="ps2")
                        for dx in range(3):
                            for gi, k in enumerate(grp):
                                m = k // 2 if ph == 0 else (k + 1) // 2
                                nc.tensor.matmul(
                                    ps2[:, 512 * gi:512 * gi + 384],
                                    w2t_sb[dx][:, :],
                                    src[:, 16 * H:16 * H + 16, m,
                                        dx:dx + W2OUT],
                                    start=(dx == 0), stop=(dx == 2))
                        # ---- fused pool eviction
                        pse = ps2.rearrange("p (g r) -> p g r", g=2)
                        pse = pse[:, :, 0:384].rearrange(
                            "p g (i x2 two) -> p g i x2 two", x2=P2, two=2)
                        odd = ep.tile([128, 384], BF16, tag="odd")
                        oddv = odd.rearrange("p (g i x) -> p g i x", g=2, x=P2)
                        nc.scalar.activation(
                            oddv[:, :, :, :], pse[:, :, :, :, 1],
                            ACTF.Relu, bias=b2v_sb[:, :])
                        ev = ep.tile([128, 384], BF16, tag="ev")
                        evv = ev.rearrange("p (g i x) -> p g i x", g=2, x=P2)
                        nc.vector.scalar_tensor_tensor(
                            out=evv[:, :, :, :], in0=pse[:, :, :, :, 0],
                            scalar=b2v_sb[:, :], in1=oddv[:, :, :, :],
                            op0=ALU.add, op1=ALU.max)
                        evhi = ep.tile([64, 384], BF16, tag="evhi")
                        nc.vector.tensor_copy(evhi[:, :], ev[64:128, :])
                        ehv = evhi.rearrange("p (g i x) -> p g i x", g=2, x=P2)
                        Yh0 = grp[0] // 2
                        dst = pooled_v[64 * ph:64 * ph + 64,
                                       i0 + 16 * H:i0 + 16 * H + 16,
                                       12 * Yh0:12 * Yh0 + 24]
                        dstv = dst.rearrange("p i (g x) -> p g i x", x=P2)
                        nc.vector.tensor_max(
                            dstv[:, :, :, :], evv[0:64, :, :, :],
                            ehv[:, :, :, :])

                # ---- software-pipelined chunk loop
                x3 = load_x3(0)
                h1 = alloc_h1()
                conv1_blocks(*x3, h1[0], range(7))
                copy_phase(*h1)
                for c in range(NCHUNK):
                    conv2_half(c, *h1, 0, [0, 2])
                    conv2_half(c, *h1, 0, [4, 6])
                    conv2_half(c, *h1, 0, [8, 10])
                    if c + 1 < NCHUNK:
                        x3n = load_x3(c + 1)
                        h1n = alloc_h1()
                        conv1_blocks(*x3n, h1n[0], range(0, 4))
                    conv2_half(c, *h1, 1, [1, 3])
                    if c + 1 < NCHUNK:
                        conv1_blocks(*x3n, h1n[0], range(4, 7))
                        copy_phase(*h1n)
                    conv2_half(c, *h1, 1, [5, 7])
                    conv2_half(c, *h1, 1, [9, 11])
                    if c + 1 < NCHUNK:
                        h1 = h1n

            # ---- fc1 (K=128 via Y-parity partitions) + fc2
            fc1l_v = fc1l_sb.rearrange("p (q m) -> p q m", m=128)
            with tc.tile_pool(name="fc_sb", bufs=1) as fp, \
                 tc.tile_pool(name="fc_ps", bufs=1, space="PSUM") as fpp:
                psF = fpp.tile([128, B_CORE], F32, tag="psF")
                for qp in range(72):
                    nc.tensor.matmul(
                        psF[:, :], fc1l_v[:, qp, :], pooled_v[:, :, qp],
                        start=(qp == 0), stop=(qp == 71))
                fc1o = fp.tile([128, B_CORE], BF16, tag="fc1o")
                nc.scalar.activation(fc1o[:, :], psF[:, :], ACTF.Relu,
                                     bias=fc1b_sb[:, :])
                psL = fpp.tile([10, B_CORE], F32, tag="psL")
                nc.tensor.matmul(psL[:, :], fc2l_sb[:, :], fc1o[:, :],
                                 start=True, stop=True)
                logit = fp.tile([10, B_CORE], F32, tag="logit")
                nc.scalar.activation(logit[:, :], psL[:, :], ACTF.Identity,
                                     bias=fc2b_sb[:, :])
                nc.sync.dma_start(out=yout[:, :], in_=logit[:, :])

    nc.compile()
    return nc


def _prep_weights(w1, b1, w2, b2, fc1_w, fc1_b, fc2_w, fc2_b):
    # conv1 lhsT, replicated in each partition quadrant:
    # w1b4[32s + 6dx + r, 32j + oc] = w1[oc, r-j, dx]
    w1b4 = np.zeros((128, 128), np.float32)
    for s in range(4):
        for r in range(6):
            for dx in range(3):
                for j in range(4):
                    dy = r - j
                    if 0 <= dy <= 2:
                        w1b4[32 * s + 6 * dx + r, 32 * j:32 * j + 32] = \
                            w1[:, 0, dy, dx]

    # conv2 pair-Toeplitz: w2t[dx][32q+ic, 64t+oc] = w2[oc, ic, q-t, dx]
    w2tl = []
    for dx in range(3):
        mth = np.zeros((128, 128), np.float32)
        for q in range(4):
            for t in range(2):
                dy = q - t
                if 0 <= dy <= 2:
                    mth[32 * q:32 * q + 32, 64 * t:64 * t + 64] = \
                        w2[:, :, dy, dx].T
        w2tl.append(mth)

    # fc1 lhsT: fc1l[64p+oc, (Yh*12+x)*128 + m] = fc1_w[m, oc*144+(2Yh+p)*12+x]
    fc1w3 = fc1_w.reshape(128, 64, 12, 12)          # m, oc, y, x
    arr = fc1w3.transpose(1, 2, 3, 0)               # oc, y, x, m
    arr = arr.reshape(64, 6, 2, 12, 128)            # oc, Yh, p, x, m
    fc1l = arr.transpose(2, 0, 1, 3, 4).reshape(128, 9216)

    bf = ml_dtypes.bfloat16
    out = {
        "w1b4": w1b4.astype(bf),
        "b1v": np.tile(b1, 4).reshape(128, 1).astype(np.float32),
        "b2v": np.tile(b2, 2).reshape(128, 1).astype(np.float32),
        "fc1l": fc1l.astype(bf),
        "fc1bv": fc1_b.reshape(128, 1).astype(np.float32),
        "fc2l": fc2_w.T.astype(bf),
        "fc2bv": fc2_b.reshape(10, 1).astype(np.float32),
    }
    for dx in range(3):
        out[f"w2t{dx}"] = w2tl[dx].astype(bf)
    return out


def _prep_x(xc):
    """Toeplitz conv1 input for one core: xc [256, 28, 28] fp32 ->
    xina/xinb [128, NCHUNK*G*26] bf16.
    Partition 32s + 6dx + r of chunk c, col i*26+x = xc[c*G+i, rows, dx+x]
    where rows = 4b + r (b = s for xina, b = s+4 for xinb)."""
    bf = ml_dtypes.bfloat16
    out = []
    for half in range(2):
        arr = np.zeros((128, NCHUNK, G, W1OUT), np.float32)
        nblk = 4 if half == 0 else 3
        for s in range(nblk):
            b = s + 4 * half
            nrow = 6 if b < 6 else 4
            for dx in range(3):
                for r in range(nrow):
                    # xc[:, 4b+r, dx:dx+26] -> [256, 26] -> chunks
                    v = xc[:, 4 * b + r, dx:dx + W1OUT]
                    arr[32 * s + 6 * dx + r] = v.reshape(NCHUNK, G, W1OUT)
        out.append(arr.reshape(128, -1).astype(bf))
    return out


def kernel(x, w1, b1, w2, b2, fc1_w, fc1_b, fc2_w, fc2_b, _trace=False):
    x = np.asarray(x, np.float32)
    wts = _prep_weights(np.asarray(w1, np.float32), np.asarray(b1, np.float32),
                        np.asarray(w2, np.float32), np.asarray(b2, np.float32),
                        np.asarray(fc1_w, np.float32), np.asarray(fc1_b, np.float32),
                        np.asarray(fc2_w, np.float32), np.asarray(fc2_b, np.float32))
    if "nc" not in _cache:
        _cache["nc"] = _build_nc()
    nc = _cache["nc"]
    in_maps = []
    for core in range(N_CORES):
        m = dict(wts)
        xa, xb = _prep_x(x[core * B_CORE:(core + 1) * B_CORE, 0])
        m["xina"] = xa
        m["xinb"] = xb
        in_maps.append(m)
    kw = {}
    if _trace:
        kw = dict(trace=True, trace_cores=[0])
    res = run_bass_kernel_spmd(nc, in_maps, core_ids=list(range(N_CORES)), **kw)
    out = np.concatenate([r["yout"].T for r in res.results], axis=0)
    _cache["last_result"] = res
    return out.astype(np.float32)


# revision 11
# speedup vs baseline: 1.5258x; 1.0053x over previous
"""Trainium2 Bass kernel for an MNIST-style CNN (conv1->relu->conv2->relu->
maxpool2x2->fc1->relu->fc2), data-parallel over 8 NeuronCores.

Per-core mapping (256 images, 8 chunks of 32):
 - x is pre-Toeplitzed on the host into [128=(blk%4, dx, row), chunk, img*26]
   bf16 so each chunk's conv1 input is ONE contiguous HWDGE DMA.
 - conv1: K=18 matmuls, blocks quadrant-aligned, 4-way row-tiled.
 - h1 stored mod-4-ring [(j=y%4, ic), (img, yblk, x)]; a phase-shifted copy
   h1k [(j2=(y+2)%4, ic), ...] is built by SBUF->SBUF DMA so every pool-pair
   of output rows is K=128-aligned.
 - conv2: per pool-pair k, 3 accumulating full-array matmuls (one per dx)
   with block-Toeplitz lhsT [128=(q,ic), 128=(t,oc)]; even pairs read h1j,
   odd pairs read h1k.
 - Software-pipelined emission: chunk c+1's conv1 (and the phase copy) is
   emitted between chunk c's even-pair and odd-pair conv2 groups, so the
   PE stream stays dense (HAM warm) and the copy DMA has a full chunk of
   slack before its consumers.
 - pool fused into eviction: ACT does odd-x (ReLU+bias), DVE STT does
   even-x max, DVE tensor_max folds the row pair across partition halves
   into pooled[(Y%2, oc), img, (Y//2, x)] (natural layout, no transposes).
 - fc1: K=128=(Y-parity, oc), 72 accumulating matmuls of N=256; fc2 plain.
"""

import os
import numpy as np
import ml_dtypes

import concourse.bass as bass
import concourse.mybir as mybir
from concourse import bacc
from concourse.tile import TileContext
from concourse.bass_utils import run_bass_kernel_spmd

BF16 = mybir.dt.bfloat16
F32 = mybir.dt.float32
ALU = mybir.AluOpType
ACTF = mybir.ActivationFunctionType

N_CORES = 8
B_CORE = 256
G = 32                      # images per chunk
NCHUNK = B_CORE // G
W1OUT = 26
W2OUT = 24
P2 = 12

_cache = {}


def _build_nc():
    n_dve_evict = int(os.environ.get("K_DVE_EVICT", "0"))
    nc = bacc.Bacc("TRN2", target_bir_lowering=False, debug=False,
                   num_devices=N_CORES)

    xina = nc.dram_tensor("xina", [128, NCHUNK * G * W1OUT], BF16,
                          kind="ExternalInput")
    xinb = nc.dram_tensor("xinb", [128, NCHUNK * G * W1OUT], BF16,
                          kind="ExternalInput")
    w1b4 = nc.dram_tensor("w1b4", [128, 128], BF16, kind="ExternalInput")
    w2t = [nc.dram_tensor(f"w2t{dx}", [128, 128], BF16, kind="ExternalInput")
           for dx in range(3)]
    b1v = nc.dram_tensor("b1v", [128, 1], F32, kind="ExternalInput")
    b2v = nc.dram_tensor("b2v", [128, 1], F32, kind="ExternalInput")
    fc1l = nc.dram_tensor("fc1l", [128, 9216], BF16, kind="ExternalInput")
    fc1bv = nc.dram_tensor("fc1bv", [128, 1], F32, kind="ExternalInput")
    fc2l = nc.dram_tensor("fc2l", [128, 10], BF16, kind="ExternalInput")
    fc2bv = nc.dram_tensor("fc2bv", [10, 1], F32, kind="ExternalInput")
    yout = nc.dram_tensor("yout", [10, B_CORE], F32, kind="ExternalOutput")

    xina_v = xina.rearrange("p (c q) -> p c q", c=NCHUNK)
    xinb_v = xinb.rearrange("p (c q) -> p c q", c=NCHUNK)

    with TileContext(nc) as tc:
        with tc.tile_pool(name="wpool", bufs=1) as wp, \
             tc.tile_pool(name="persist", bufs=1) as pers:
            def load_w(dram, shape, dtype=BF16, tag=None):
                t = wp.tile(shape, dtype, tag=tag or dram.name)
                nc.sync.dma_start(out=t[:, :], in_=dram[:, :])
                return t

            w1b4_sb = load_w(w1b4, [128, 128])
            w2t_sb = [load_w(d, [128, 128]) for d in w2t]
            b1v_sb = load_w(b1v, [128, 1], F32)
            b2v_sb = load_w(b2v, [128, 1], F32)
            fc1l_sb = load_w(fc1l, [128, 9216])
            fc1b_sb = load_w(fc1bv, [128, 1], F32)
            fc2l_sb = load_w(fc2l, [128, 10])
            fc2b_sb = load_w(fc2bv, [10, 1], F32)

            # pooled[(Y%2)*64+oc, (img, qp=(Y//2)*12+x)]
            pooled = pers.tile([128, B_CORE * 72], BF16, tag="pooled")
            pooled_v = pooled.rearrange("p (i q) -> p i q", q=72)

            with tc.tile_pool(name="x3_sb", bufs=2) as cp, \
                 tc.tile_pool(name="h1_sb", bufs=2) as hp, \
                 tc.tile_pool(name="evict_sb", bufs=4) as ep, \
                 tc.tile_pool(name="ps1", bufs=2, space="PSUM") as pp1, \
                 tc.tile_pool(name="ps2", bufs=3, space="PSUM") as pp2:

                def load_x3(c):
                    x3a = cp.tile([128, G * W1OUT], BF16, tag="x3a")
                    x3b = cp.tile([128, G * W1OUT], BF16, tag="x3b")
                    nc.sync.dma_start(out=x3a[:, :], in_=xina_v[:, c, :])
                    nc.sync.dma_start(out=x3b[:, :], in_=xinb_v[:, c, :])
                    return x3a, x3b

                def alloc_h1():
                    h1j = hp.tile([128, G * 7 * W1OUT], BF16, tag="h1j")
                    h1k = hp.tile([128, G * 7 * W1OUT], BF16, tag="h1k")
                    h1v = h1j.rearrange("p (i y x) -> p i y x", y=7, x=W1OUT)
                    h1kv = h1k.rearrange("p (i y x) -> p i y x", y=7, x=W1OUT)
                    return h1v, h1kv

                def conv1_blocks(x3a, x3b, h1v, blocks):
                    ne = 0
                    for H in range(2):
                        for b in blocks:
                            t3 = (x3a if b < 4 else x3b)
                            s = b % 4
                            kk = 18 if b < 6 else 16
                            mm = 128 if b < 6 else 64
                            x3v = t3[32 * s:32 * s + kk, :].rearrange(
                                "k (i x) -> k i x", x=W1OUT)
                            ps1 = pp1.tile([128, 16 * W1OUT], F32, tag="ps1")
                            nc.tensor.matmul(
                                ps1[0:mm, :],
                                w1b4_sb[32 * s:32 * s + kk, 0:mm],
                                x3v[:, 16 * H:16 * H + 16, :],
                                start=True, stop=True,
                                tile_position=(32 * s, 0))
                            dst = h1v[0:mm, 16 * H:16 * H + 16, b, :]
                            if ne < n_dve_evict:
                                nc.vector.tensor_scalar(
                                    out=dst, in0=ps1[0:mm, :],
                                    scalar1=b1v_sb[0:mm, :], scalar2=0.0,
                                    op0=ALU.add, op1=ALU.max)
                            else:
                                nc.scalar.activation(
                                    dst, ps1[0:mm, :], ACTF.Relu,
                                    bias=b1v_sb[0:mm, :])
                            ne += 1

                def copy_phase(h1v, h1kv):
                    # SBUF->SBUF DMA on the sync HWDGE ring
                    nc.sync.dma_start(
                        out=h1kv[0:64, :, 1:7, :], in_=h1v[64:128, :, 0:6, :])
                    nc.sync.dma_start(
                        out=h1kv[64:128, :, 0:7, :], in_=h1v[0:64, :, 0:7, :])

                def conv2_half(c, h1v, h1kv, ph, grp):
                    i0 = c * G
                    src = h1v if ph == 0 else h1kv
                    for H in range(2):
                        ps2 = pp2.tile([128, 1024], F32, tag="ps2")
                        for dx in range(3):
                            for gi, k in enumerate(grp):
                                m = k // 2 if ph == 0 else (k + 1) // 2
                                nc.tensor.matmul(
                                    ps2[:, 512 * gi:512 * gi + 384],
                                    w2t_sb[dx][:, :],
                                    src[:, 16 * H:16 * H + 16, m,
                                        dx:dx + W2OUT],
                                    start=(dx == 0), stop=(dx == 2))
                        # ---- fused pool eviction
                        pse = ps2.rearrange("p (g r) -> p g r", g=2)
                        pse = pse[:, :, 0:384].rearrange(
                            "p g (i x2 two) -> p g i x2 two", x2=P2, two=2)
                        odd = ep.tile([128, 384], BF16, tag="odd")
                        oddv = odd.rearrange("p (g i x) -> p g i x", g=2, x=P2)
                        nc.scalar.activation(
                            oddv[:, :, :, :], pse[:, :, :, :, 1],
                            ACTF.Relu, bias=b2v_sb[:, :])
                        ev = ep.tile([128, 384], BF16, tag="ev")
                        evv = ev.rearrange("p (g i x) -> p g i x", g=2, x=P2)
                        nc.vector.scalar_tensor_tensor(
                            out=evv[:, :, :, :], in0=pse[:, :, :, :, 0],
                            scalar=b2v_sb[:, :], in1=oddv[:, :, :, :],
                            op0=ALU.add, op1=ALU.max)
                        evhi = ep.tile([64, 384], BF16, tag="evhi")
                        nc.vector.tensor_copy(evhi[:, :], ev[64:128, :])
                        ehv = evhi.rearrange("p (g i x) -> p g i x", g=2, x=P2)
                        Yh0 = grp[0] // 2
                        dst = pooled_v[64 * ph:64 * ph + 64,
                                       i0 + 16 * H:i0 + 16 * H + 16,
                                       12 * Yh0:12 * Yh0 + 24]
                        dstv = dst.rearrange("p i (g x) -> p g i x", x=P2)
                        nc.vector.tensor_max(
                            dstv[:, :, :, :], evv[0:64, :, :, :],
                            ehv[:, :, :, :])

                # ---- software-pipelined chunk loop
                x3 = load_x3(0)
                h1 = alloc_h1()
                conv1_blocks(*x3, h1[0], range(7))
                copy_phase(*h1)
                for c in range(NCHUNK):
                    if c + 1 < NCHUNK:
                        x3n = load_x3(c + 1)
                        h1n = alloc_h1()
                    conv2_half(c, *h1, 0, [0, 2])
                    conv2_half(c, *h1, 0, [4, 6])
                    conv2_half(c, *h1, 0, [8, 10])
                    if c + 1 < NCHUNK:
                        conv1_blocks(*x3n, h1n[0], range(0, 4))
                    conv2_half(c, *h1, 1, [1, 3])
                    if c + 1 < NCHUNK:
                        conv1_blocks(*x3n, h1n[0], range(4, 7))
                        copy_phase(*h1n)
                    conv2_half(c, *h1, 1, [5, 7])
                    conv2_half(c, *h1, 1, [9, 11])
                    if c + 1 < NCHUNK:
                        h1 = h1n

            # ---- fc1 (K=128 via Y-parity partitions) + fc2
            fc1l_v = fc1l_sb.rearrange("p (q m) -> p q m", m=128)
            with tc.tile_pool(name="fc_sb", bufs=1) as fp, \
                 tc.tile_pool(name="fc_ps", bufs=1, space="PSUM") as fpp:
                psF = fpp.tile([128, B_CORE], F32, tag="psF")
                for qp in range(72):
                    nc.tensor.matmul(
                        psF[:, :], fc1l_v[:, qp, :], pooled_v[:, :, qp],
                        start=(qp == 0), stop=(qp == 71))
                fc1o = fp.tile([128, B_CORE], BF16, tag="fc1o")
                nc.scalar.activation(fc1o[:, :], psF[:, :], ACTF.Relu,
                                     bias=fc1b_sb[:, :])
                psL = fpp.tile([10, B_CORE], F32, tag="psL")
                nc.tensor.matmul(psL[:, :], fc2l_sb[:, :], fc1o[:, :],
                                 start=True, stop=True)
                logit = fp.tile([10, B_CORE], F32, tag="logit")
                nc.scalar.activation(logit[:, :], psL[:, :], ACTF.Identity,
                                     bias=fc2b_sb[:, :])
                nc.sync.dma_start(out=yout[:, :], in_=logit[:, :])

    nc.compile()
    return nc


def _prep_weights(w1, b1, w2, b2, fc1_w, fc1_b, fc2_w, fc2_b):
    # conv1 lhsT, replicated in each partition quadrant:
    # w1b4[32s + 6dx + r, 32j + oc] = w1[oc, r-j, dx]
    w1b4 = np.zeros((128, 128), np.float32)
    for s in range(4):
        for r in range(6):
            for dx in range(3):
                for j in range(4):
                    dy = r - j
                    if 0 <= dy <= 2:
                        w1b4[32 * s + 6 * dx + r, 32 * j:32 * j + 32] = \
                            w1[:, 0, dy, dx]

    # conv2 pair-Toeplitz: w2t[dx][32q+ic, 64t+oc] = w2[oc, ic, q-t, dx]
    w2tl = []
    for dx in range(3):
        mth = np.zeros((128, 128), np.float32)
        for q in range(4):
            for t in range(2):
                dy = q - t
                if 0 <= dy <= 2:
                    mth[32 * q:32 * q + 32, 64 * t:64 * t + 64] = \
                        w2[:, :, dy, dx].T
        w2tl.append(mth)

    # fc1 lhsT: fc1l[64p+oc, (Yh*12+x)*128 + m] = fc1_w[m, oc*144+(2Yh+p)*12+x]
    fc1w3 = fc1_w.reshape(128, 64, 12, 12)          # m, oc, y, x
    arr = fc1w3.transpose(1, 2, 3, 0)               # oc, y, x, m
    arr = arr.reshape(64, 6, 2, 12, 128)            # oc, Yh, p, x, m
    fc1l = arr.transpose(2, 0, 1, 3, 4).reshape(128, 9216)

    bf = ml_dtypes.bfloat16
    out = {
        "w1b4": w1b4.astype(bf),
        "b1v": np.tile(b1, 4).reshape(128, 1).astype(np.float32),
        "b2v": np.tile(b2, 2).reshape(128, 1).astype(np.float32),
        "fc1l": fc1l.astype(bf),
        "fc1bv": fc1_b.reshape(128, 1).astype(np.float32),
        "fc2l": fc2_w.T.astype(bf),
        "fc2bv": fc2_b.reshape(10, 1).astype(np.float32),
    }
    for dx in range(3):
        out[f"w2t{dx}"] = w2tl[dx].astype(bf)
    return out


def _prep_x(xc):
    """Toeplitz conv1 input for one core: xc [256, 28, 28] fp32 ->
    xina/xinb [128, NCHUNK*G*26] bf16.
    Partition 32s + 6dx + r of chunk c, col i*26+x = xc[c*G+i, rows, dx+x]
    where rows = 4b + r (b = s for xina, b = s+4 for xinb)."""
    bf = ml_dtypes.bfloat16
    out = []
    for half in range(2):
        arr = np.zeros((128, NCHUNK, G, W1OUT), np.float32)
        nblk = 4 if half == 0 else 3
        for s in range(nblk):
            b = s + 4 * half
            nrow = 6 if b < 6 else 4
            for dx in range(3):
                for r in range(nrow):
                    # xc[:, 4b+r, dx:dx+26] -> [256, 26] -> chunks
                    v = xc[:, 4 * b + r, dx:dx + W1OUT]
                    arr[32 * s + 6 * dx + r] = v.reshape(NCHUNK, G, W1OUT)
        out.append(arr.reshape(128, -1).astype(bf))
    return out


def kernel(x, w1, b1, w2, b2, fc1_w, fc1_b, fc2_w, fc2_b, _trace=False):
    x = np.asarray(x, np.float32)
    wts = _prep_weights(np.asarray(w1, np.float32), np.asarray(b1, np.float32),
                        np.asarray(w2, np.float32), np.asarray(b2, np.float32),
                        np.asarray(fc1_w, np.float32), np.asarray(fc1_b, np.float32),
                        np.asarray(fc2_w, np.float32), np.asarray(fc2_b, np.float32))
    if "nc" not in _cache:
        _cache["nc"] = _build_nc()
    nc = _cache["nc"]
    in_maps = []
    for core in range(N_CORES):
        m = dict(wts)
        xa, xb = _prep_x(x[core * B_CORE:(core + 1) * B_CORE, 0])
        m["xina"] = xa
        m["xinb"] = xb
        in_maps.append(m)
    kw = {}
    if _trace:
        kw = dict(trace=True, trace_cores=[0])
    res = run_bass_kernel_spmd(nc, in_maps, core_ids=list(range(N_CORES)), **kw)
    out = np.concatenate([r["yout"].T for r in res.results], axis=0)
    _cache["last_result"] = res
    return out.astype(np.float32)
